# revision 1
# baseline (speedup 1.0000x reference)
"""Trainium2 Bass kernel for CausalGraphAttention (N=8192, F=256), 8-core SPMD.

Math (per reference):
  h      = x @ W                               [N, F]
  e[i,j] = leaky_relu(h[i]@a1 + h[j]@a2, 0.2)
           + (cs[j] - cs[i]) * cw[i,j],   cs = x @ c
  e      = where(adj, e, -9e15);  att = softmax(e, axis=1);  out = att @ h

Device strategy (1D row-parallel, transposed score layout):
  - Each core owns a 1024-row block of the score matrix. All score tiles are
    computed TRANSPOSED: eT[j, i] with j on partitions, i on the free dim, so
    the final contraction over j maps directly onto the tensor engine
    (lhsT = exp(eT) subtile [j,128i], rhs = [h | 1 | 1] tile [j,258]) and
    the softmax denominator falls out of the ones columns of the rhs for free.

  - Elementwise restructuring (vs a naive 4-pass pipeline):
      * softmax row-invariance drops the exp(ss_i) factor entirely;
      * exp(leaky(u)) = exp(u)*max(exp(-0.8u),1) makes the LeakyReLU factor
        separable: with exp monotone,
          p = exp( max((cs_j-cs_i)*M, -30)                  [W3]
                 + max(-0.8*ss_i + 0.2*sd_j, sd_j)          [LADD]
                 - 4 )                                      [exp bias]
      * the adjacency mask is encoded as NaN in the fp16 causal-weight
        matrix: the DVE ALU's MAX suppresses NaN (returns the non-NaN
        operand), so W3 maps non-edges to exactly -30 at zero extra cost.
    W3 and LADD are 3-slice custom DVE ops with hand-written 2x_1p packed
    uop variants (2 fp16 elements/cycle); exp is quad-batched on ScalarE.
  - All per-j scalars (sd, 0.2*sd, cs) fall out of the h-pass matmul via
    W_aug = [W@a2 | 0.2*W@a2 | c | W]; replicated-weight matmuls give the
    per-i row vectors (-cs, -0.8*ss) broadcast across all 128 partitions.
  - The h pass is interleaved with the main loop in 2-j-tile groups so the
    elementwise/exp/matmul pipeline starts ~15us into the kernel instead of
    waiting for the full h pass (ScalarE's queue is strict FIFO: the h
    copies would otherwise block the first exp for ~35us).
"""

import numpy as np

import concourse.mybir as mybir
import concourse.tile as tile
from concourse import bacc
from concourse import dve_ops as _dops
from concourse.bass_utils import run_bass_kernel_spmd
from concourse.dve_ops import DveOp, get_dve_sub_opcode
from concourse.dve_spec import C0, C1, Spec, Src0, Src1, _has_src1, lower, maxx
from concourse.dve_uop import (ENABLE, AluInp, AluOp, DelayInp, DveOpSpec,
                               InpSel, OutPath, OutSel, Trigger, UopConfig,
                               UopDpConfig)

dt = mybir.dt
AF = mybir.ActivationFunctionType
ALU = mybir.AluOpType

N = 8192
F = 256
NCORES = 8
RPC = N // NCORES          # rows per core (i range)
NJT = N // 128             # j tiles of 128
NSUB = RPC // 128          # i subtiles of 128
MASK_NEG = -30.0           # masked logits: e = -30 + L - 4 -> exp ~ 0
HSTRIDE = F + 4            # h tile layout: [h(256) | ones(2) | pad(2)], 8B-aligned
NMM = F + 2                # matmul rhs width: [h | 1 | 1] (even for full-rate streaming)
NAUG = F + 3               # h-pass rhs width: [sd | 0.2*sd | cs | h]


def _packed_2x_uop(cfg):
    """Hand-written 2x_1p packed uop: stages 0-2 run the 3-op body on the lo
    half of each packed fp16 pair, stages 3-5 on the hi half (via SRC_*_HI),
    the lo result is captured into delay lane 5 at stage 3 and both results
    are emitted packed through WR0_LO/WR0_HI."""
    u = UopConfig()
    for sel, slot in cfg["inputs"]:
        u.enable_input(sel, slot)
    lanes = (0, 1, 2, 3, 4)
    dp = [UopDpConfig() for _ in range(8)]
    for st, (op, a, b) in enumerate(cfg["stages"]):
        dp[st].enable_alu(op, a, b)
        dp[st].pass_through_delay(*(lanes if st < 4 else (*lanes, 5)))
    dp[3].enable_delay_from_src(DelayInp.PREV_ALU_OUT, 5)
    for st in range(len(cfg["stages"]), 8):
        dp[st].enable_alu(AluOp.BYPASS, AluInp.PREV_ALU_OUT, AluInp.PREV_ALU_OUT)
        dp[st].pass_through_delay(*lanes, 5)
    u.datapath_config = dp
    u.enable_output(OutSel.DELAY_5, OutPath.WR0_LO)
    u.enable_output(OutSel.ALU_OUT, OutPath.WR0_HI)
    u.require_inp0 = ENABLE
    u.require_inp1 = ENABLE
    u.trigger = (Trigger.SRC_TENSOR_DONE, Trigger.NONE, Trigger.NONE)
    u.next_uop = (0, 0, 0)
    return u


_PAL = AluInp.PREV_ALU_OUT
_D = (AluInp.PREV_DELAY_0, AluInp.PREV_DELAY_1, AluInp.PREV_DELAY_2,
      AluInp.PREV_DELAY_3, AluInp.PREV_DELAY_4, AluInp.PREV_DELAY_5)


def _w3_2x_uop():
    # w = max((Src1 + C0) * Src0, C1); Src1 lo enters via inp slot 0.
    return _packed_2x_uop({
        "inputs": [(InpSel.SRC_1, 0), (InpSel.CONST_0, 1), (InpSel.SRC_0, 2),
                   (InpSel.CONST_1, 3), (InpSel.SRC_1_HI, 4), (InpSel.SRC_0_HI, 5)],
        "stages": [
            (AluOp.ADD, _PAL, _D[0]),        # negcs_lo + cs_j
            (AluOp.MULTIPLY, _PAL, _D[1]),   # * M_lo
            (AluOp.MAX, _PAL, _D[2]),        # max(, -30)
            (AluOp.ADD, _D[3], _D[0]),       # negcs_hi + cs_j   (+ lo capture)
            (AluOp.MULTIPLY, _PAL, _D[4]),   # * M_hi
            (AluOp.MAX, _PAL, _D[2]),        # max(, -30)
        ],
    })


def _ladd_2x_uop():
    # e = max(Src0 + C0, C1) + Src1; Src0 lo enters via inp slot 0.
    return _packed_2x_uop({
        "inputs": [(InpSel.SRC_0, 0), (InpSel.CONST_0, 1), (InpSel.CONST_1, 2),
                   (InpSel.SRC_1, 3), (InpSel.SRC_0_HI, 4), (InpSel.SRC_1_HI, 5)],
        "stages": [
            (AluOp.ADD, _PAL, _D[0]),        # ss2_lo + c1
            (AluOp.MAX, _PAL, _D[1]),        # max(, c2)
            (AluOp.ADD, _PAL, _D[2]),        # + w_lo
            (AluOp.ADD, _D[3], _D[0]),       # ss2_hi + c1       (+ lo capture)
            (AluOp.MAX, _PAL, _D[1]),        # max(, c2)
            (AluOp.ADD, _PAL, _D[4]),        # + w_hi
        ],
    })


_UOP2X_BUILDERS = {"CGA_W3": _w3_2x_uop, "CGA_LADD": _ladd_2x_uop}


class DveOp2x(DveOp):
    """DveOp whose compiled table also carries a hand-written 2x_1p uop
    program; emitted instructions additionally set perf_max=1 so the
    engine may select the packed mode when dtype/stride conditions hold."""

    def compile(self, ver):
        key = ("2x:" + self.name, ver)
        cached = _dops._COMPILE_CACHE.get(key)
        if cached is not None:
            return cached
        result = DveOpSpec(
            name=self.name,
            opcode=get_dve_sub_opcode(self.name),
            uops=lower(self.spec, ver=ver),
            rd1_en=_has_src1(self.spec),
            uops_2x=[_UOP2X_BUILDERS[self.name]()] if ver == "v3" else None,
            perf_max=1 if ver == "v3" else 0,
        )
        _dops._COMPILE_CACHE[key] = result
        return result


def _register(name, spec):
    for op in _dops.OPS:
        if op.name == name:
            return op
    opcode = _dops._CUSTOM_DVE_ROW_BASE + len(_dops.OPS)
    assert opcode < 0x20
    _dops._SUB_OPCODE_FOR_NAME[name] = opcode
    shas = {}
    for ver in ("v3", "v4"):
        s = DveOpSpec(name=name, opcode=opcode, uops=lower(spec, ver=ver),
                      rd1_en=_has_src1(spec))
        shas[ver] = s.sha(ver)
    op = DveOp2x(name, spec, subdim=False, uops_sha=shas)
    _dops.OPS.append(op)
    _dops.CUSTOM_DVE_SPECS[name] = op.spec
    return op


# w = max((negcs + cs_j) * M, -30): causal product + NaN-encoded mask
W3_OP = _register("CGA_W3", Spec(
    body=maxx((Src1 + C0) * Src0, C1),
    reference=lambda in0, in1, s0, s1: np.fmax((in1 + s0) * in0, s1)))

# e = max(ss2 + c1_j, c2_j) + w: leaky factor + add, one pass
LADD_OP = _register("CGA_LADD", Spec(
    body=maxx(Src0 + C0, C1) + Src1,
    reference=lambda in0, in1, s0, s1: np.maximum(in0 + s0, s1) + in1))


def _emit2x(nc, op, out, in0, in1, s0, s1):
    bi = nc.vector._custom_dve(op, out=out, in0=in0, in1=in1, s0=s0, s1=s1)
    bi.ins.perf_max = 1
    return bi


def build_program():
    nc = bacc.Bacc("TRN2", target_bir_lowering=False, debug=False,
                   num_devices=NCORES)

    xT = nc.declare_dram_parameter("xT", [F, N], dt.float16, isOutput=False)
    xTown = nc.declare_dram_parameter("xTown", [F, RPC], dt.float16, isOutput=False)
    Waug = nc.declare_dram_parameter("Waug", [F, NAUG], dt.float16, isOutput=False)
    WA1rep = nc.declare_dram_parameter("WA1rep", [F, 128], dt.float16, isOutput=False)
    WCnegrep = nc.declare_dram_parameter("WCnegrep", [F, 128], dt.float16, isOutput=False)
    cwmT = nc.declare_dram_parameter("cwmT", [N, RPC], dt.float16, isOutput=False)
    out_d = nc.declare_dram_parameter("out", [RPC, F], dt.float32, isOutput=True)
    dbg_d = nc.declare_dram_parameter("dbg", [128, 256], dt.float16, isOutput=True)

    with tile.TileContext(nc) as tc:
        with (
            tc.tile_pool(name="persist", bufs=1) as persist,
            tc.tile_pool(name="main", bufs=2) as main_pool,
            tc.tile_pool(name="tail", bufs=2) as tailp,
        ):
            # --- persistent tiles ---
            h_all = persist.tile([128, NJT * HSTRIDE], dt.bfloat16, tag="h_all")
            scols = persist.tile([128, 3 * NJT], dt.float32, tag="scols")
            ss2_repl = persist.tile([128, RPC], dt.float16, tag="ss2_repl")
            negcs_repl = persist.tile([128, RPC], dt.float16, tag="negcs_repl")
            waug_sb = persist.tile([128, 2, NAUG], dt.float16, tag="waug")
            wa1_sb = persist.tile([128, 2, 128], dt.float16, tag="wa1")
            wcn_sb = persist.tile([128, 2, 128], dt.float16, tag="wcn")
            xtown_sb = persist.tile([128, 2, RPC], dt.float16, tag="xtown")
            dbg_sb = persist.tile([128, 256], dt.float16, tag="dbg")

            nc.sync.dma_start(out=xtown_sb[:], in_=xTown.ap().rearrange("(b p) f -> p b f", p=128))
            nc.sync.dma_start(out=wa1_sb[:], in_=WA1rep.ap().rearrange("(b p) f -> p b f", p=128))
            nc.sync.dma_start(out=wcn_sb[:], in_=WCnegrep.ap().rearrange("(b p) f -> p b f", p=128))
            nc.sync.dma_start(out=waug_sb[:], in_=Waug.ap().rearrange("(b p) f -> p b f", p=128))

            cw_src = cwmT.ap().rearrange("(c p) i -> p c i", p=128)
            cw_tiles = {}

            def fetch_cw(jq):
                if jq not in cw_tiles and jq < NJT // 4:
                    t = main_pool.tile([128, 4, RPC], dt.float16, tag="cw",
                                       bufs=4, name=f"cw{jq}")
                    nc.sync.dma_start(out=t[:], in_=cw_src[:, 4 * jq:4 * jq + 4, :])
                    cw_tiles[jq] = t
                return cw_tiles.get(jq)

            ones_ap = h_all[:].rearrange("p (t c) -> p t c", c=HSTRIDE)[:, :, F:F + 2]
            nc.vector.memset(ones_ap, 1.0)
            exp_bias = persist.tile([128, 1], dt.float32, tag="exp_bias")
            nc.vector.memset(exp_bias[:], -4.0)

            h_view = h_all[:].rearrange("p (t c) -> p t c", c=HSTRIDE)
            sc_view = scols[:].rearrange("p (t c) -> p t c", c=3)

            with (
                tc.tile_pool(name="xt_pool", bufs=1) as xt_pool,
            ):
                xt_sb = xt_pool.tile([128, 2, N], dt.float16, tag="xt")
                xt_src = xT.ap().rearrange("(b p) f -> p b f", p=128)
                # the first causal-weight chunks gate the elementwise
                # pipeline start; request them before the bulk xT fetch
                fetch_cw(0)
                fetch_cw(1)
                for blk in range(8):
                    sl = slice(blk * (N // 8), (blk + 1) * (N // 8))
                    nc.sync.dma_start(out=xt_sb[:, :, sl], in_=xt_src[:, :, sl])

                # replicated -0.8*ss[i] and -cs[i] across all partitions
                with tc.tile_pool(name="psum_s", bufs=2, space="PSUM") as psum_s:
                    for (w_sb, dest) in ((wcn_sb, negcs_repl), (wa1_sb, ss2_repl)):
                        for half in range(RPC // 512):
                            ps = psum_s.tile([128, 512], dt.float32, tag="ps_s")
                            for kh in range(2):
                                nc.tensor.matmul(
                                    ps[:], lhsT=w_sb[:, kh, :],
                                    rhs=xtown_sb[:, kh, half * 512:(half + 1) * 512],
                                    start=(kh == 0), stop=(kh == 1))
                            if dest is negcs_repl:
                                nc.vector.tensor_copy(dest[:, half * 512:(half + 1) * 512], ps[:])
                            else:
                                nc.scalar.activation(dest[:, half * 512:(half + 1) * 512],
                                                     ps[:], AF.Copy, scale=-0.8)

                p_quads = {}

                def emit_quad_ew(jq):
                    w_quad = main_pool.tile([128, 4, RPC], dt.float16, tag="w", bufs=2)
                    e_quad = main_pool.tile([128, 4, RPC], dt.float16, tag="e", bufs=3)
                    p_quad = main_pool.tile([128, 4, RPC], dt.float16, tag="p", bufs=3)
                    p_quads[jq] = p_quad
                    cw_t = fetch_cw(jq)
                    fetch_cw(jq + 1)
                    fetch_cw(jq + 2)
                    for q in range(4):
                        jt = 4 * jq + q
                        # w = max((negcs_i + cs_j) * M, -30)   (NaN mask)
                        _emit2x(nc, W3_OP, out=w_quad[:, q, :], in0=cw_t[:, q, :],
                                in1=negcs_repl[:], s0=sc_view[:, jt, 2:3],
                                s1=MASK_NEG)
                        # e = max(ss2_i + 0.2*sd_j, sd_j) + w
                        _emit2x(nc, LADD_OP, out=e_quad[:, q, :], in0=ss2_repl[:],
                                in1=w_quad[:, q, :], s0=sc_view[:, jt, 1:2],
                                s1=sc_view[:, jt, 0:1])
                    # p = exp(e - 4) over the quad
                    nc.scalar.activation(p_quad[:], e_quad[:], AF.Exp, bias=exp_bias[:])

                # h pass; the elementwise/exp for the first 3 quads (= the
                # p-buffer depth, so no rotation hazard) is interleaved so
                # the DVE/ScalarE pipeline starts during the h pass instead
                # of behind it in the strict-FIFO engine queues
                with tc.tile_pool(name="psum_h", bufs=2, space="PSUM") as psum_h:
                    for g in range(NJT // 4):
                        # each t-slice padded to 512 fp32 = one PSUM bank
                        # (a matmul output must not cross a bank boundary)
                        ps = psum_h.tile([128, 4, 512], dt.float32, tag="ps_h")
                        for t in range(4):
                            jt = 4 * g + t
                            for kh in range(2):
                                nc.tensor.matmul(
                                    ps[:, t, 0:NAUG],
                                    lhsT=xt_sb[:, kh, jt * 128:(jt + 1) * 128],
                                    rhs=waug_sb[:, kh, :],
                                    start=(kh == 0), stop=(kh == 1))
                        nc.scalar.copy(h_view[:, 4 * g:4 * g + 4, 0:F], ps[:, :, 3:F + 3])
                        nc.vector.tensor_copy(sc_view[:, 4 * g:4 * g + 4, :], ps[:, :, 0:3])

            # self-test of both packed 2x custom-op variants
            _emit2x(nc, W3_OP, out=dbg_sb[:, 0:128], in0=cw_tiles[0][:, 0, 0:128],
                    in1=negcs_repl[:, 0:128], s0=sc_view[:, 0, 2:3], s1=MASK_NEG)
            _emit2x(nc, LADD_OP, out=dbg_sb[:, 128:256], in0=ss2_repl[:, 0:128],
                    in1=dbg_sb[:, 0:128], s0=sc_view[:, 0, 1:2],
                    s1=sc_view[:, 0, 0:1])
            nc.sync.dma_start(out=dbg_d.ap(), in_=dbg_sb[:])

            # --- main loop: 4 j-tiles per iteration (quad-batched exp) ---
            with tc.tile_pool(name="psum_o", bufs=1, space="PSUM") as psum_o:
                out_ps = [psum_o.tile([128, NMM], dt.float32, tag=f"out{s}",
                                      name=f"out_ps{s}")
                          for s in range(NSUB)]

                for jq in range(NJT // 4):
                    w_quad = main_pool.tile([128, 4, RPC], dt.float16, tag="w", bufs=2)
                    e_quad = main_pool.tile([128, 4, RPC], dt.float16, tag="e", bufs=3)
                    p_quad = main_pool.tile([128, 4, RPC], dt.float16, tag="p", bufs=3)
                    cw_t = fetch_cw(jq)
                    fetch_cw(jq + 1)
                    fetch_cw(jq + 2)
                    for q in range(4):
                        jt = 4 * jq + q
                        _emit2x(nc, W3_OP, out=w_quad[:, q, :], in0=cw_t[:, q, :],
                                in1=negcs_repl[:], s0=sc_view[:, jt, 2:3],
                                s1=MASK_NEG)
                        _emit2x(nc, LADD_OP, out=e_quad[:, q, :], in0=ss2_repl[:],
                                in1=w_quad[:, q, :], s0=sc_view[:, jt, 1:2],
                                s1=sc_view[:, jt, 0:1])
                    nc.scalar.activation(p_quad[:], e_quad[:], AF.Exp, bias=exp_bias[:])
                    # out[i, :] += p^T @ [h | 1]
                    for q in range(4):
                        jt = 4 * jq + q
                        for s in range(NSUB):
                            nc.tensor.matmul(
                                out_ps[s][:],
                                lhsT=p_quad[:, q, s * 128:(s + 1) * 128],
                                rhs=h_view[:, jt, 0:NMM],
                                start=(jt == 0), stop=(jt == NJT - 1))

                # --- tail: normalize and write out ---
                o_all = tailp.tile([128, NSUB, F], dt.float32, tag="osb", bufs=1)
                for s in range(NSUB):
                    rec = tailp.tile([128, 1], dt.float32, tag="rec", bufs=4)
                    nc.vector.reciprocal(rec[:], out_ps[s][:, F:F + 1])
                    nc.scalar.activation(o_all[:, s, :], out_ps[s][:, 0:F], AF.Copy,
                                         scale=rec[:])
                nc.sync.dma_start(out=out_d.ap().rearrange("(s p) f -> p s f", p=128),
                                  in_=o_all[:])

    nc.compile()
    return nc


_CACHED_NC = None


def _get_program():
    global _CACHED_NC
    if _CACHED_NC is None:
        _CACHED_NC = build_program()
    return _CACHED_NC


def _host_prep(x, adj, causal_weights, W, a1, a2, c):
    x = np.asarray(x, dtype=np.float32)
    adj = np.asarray(adj)
    cw = np.asarray(causal_weights, dtype=np.float32)
    W = np.asarray(W, dtype=np.float32)
    a1 = np.asarray(a1, dtype=np.float32)
    a2 = np.asarray(a2, dtype=np.float32)
    c = np.asarray(c, dtype=np.float32)

    wa1 = W @ a1
    wa2 = W @ a2
    waug = np.concatenate([wa2[:, None], 0.2 * wa2[:, None], c[:, None], W],
                          axis=1).astype(np.float16)
    wa1rep = np.repeat(wa1[:, None], 128, axis=1).astype(np.float16)
    wcnegrep = np.repeat(-c[:, None], 128, axis=1).astype(np.float16)
    xt16 = np.ascontiguousarray(x.T).astype(np.float16)

    # NaN-encoded mask: edge -> causal weight, non-edge -> NaN (the DVE MAX
    # suppresses NaN, so max(w_raw, -30) maps non-edges to exactly -30)
    cwm = np.where(adj > 0, cw, np.nan).astype(np.float16)

    in_maps = []
    for k in range(NCORES):
        r0, r1 = k * RPC, (k + 1) * RPC
        in_maps.append({
            "xT": xt16,
            "xTown": np.ascontiguousarray(xt16[:, r0:r1]),
            "Waug": waug,
            "WA1rep": wa1rep,
            "WCnegrep": wcnegrep,
            "cwmT": np.ascontiguousarray(cwm[r0:r1, :].T),
        })
    return in_maps


def kernel(x, adj, causal_weights, W, a1, a2, c, _trace=False, _trace_kwargs=None):
    nc = _get_program()
    in_maps = _host_prep(x, adj, causal_weights, W, a1, a2, c)
    kw = {}
    if _trace:
        kw["trace"] = True
        kw.update(_trace_kwargs or {})
    res = run_bass_kernel_spmd(nc, in_maps, list(range(NCORES)), **kw)
    out = np.concatenate([res.results[k]["out"] for k in range(NCORES)], axis=0)

    # host check of the on-device 2x custom-op self-tests (core 0, jt=0):
    # dbg[:, 0:128]   = fmax((-cs_i + cs_p) * M[i, p], -30)          (W3)
    # dbg[:, 128:256] = max(ss2_i + 0.2*sd_p, sd_p) + W3             (LADD)
    try:
        xf = np.asarray(x, dtype=np.float32)
        h = xf @ np.asarray(W, dtype=np.float32)
        cs = xf @ np.asarray(c, dtype=np.float32)
        ss = h @ np.asarray(a1, dtype=np.float32)
        sd = h @ np.asarray(a2, dtype=np.float32)
        cwm0 = in_maps[0]["cwmT"]  # [N, RPC]: [j, i]
        negcs16 = (-cs[:128]).astype(np.float16).astype(np.float32)
        w_exp = np.fmax((negcs16[None, :] + cs[:128, None])
                        * cwm0[:128, :128].astype(np.float32), MASK_NEG)
        ss2 = (-0.8 * ss[:128]).astype(np.float16).astype(np.float32)
        e_exp = np.maximum(ss2[None, :] + 0.2 * sd[:128, None], sd[:128, None]) + w_exp
        got = np.asarray(res.results[0]["dbg"], dtype=np.float32)
        for name, sl, exp_v in (("w3", slice(0, 128), w_exp),
                                ("ladd", slice(128, 256), e_exp)):
            diff = np.abs(got[:, sl] - exp_v)
            print(f"[{name}-2x selftest] max abs diff: {np.nanmax(diff):.4f} "
                  f"(n>0.08: {(diff > 0.08).sum()})")
    except Exception as exc:  # pragma: no cover - diagnostic only
        print(f"[2x selftest] check failed: {exc}")

    if _trace:
        return out, res
    return out



# revision 3
# speedup vs baseline: 1.0022x; 1.0022x over previous
"""Trainium2 Bass kernel for CausalGraphAttention (N=8192, F=256), 8-core SPMD.

Math (per reference):
  h      = x @ W                               [N, F]
  e[i,j] = leaky_relu(h[i]@a1 + h[j]@a2, 0.2)
           + (cs[j] - cs[i]) * cw[i,j],   cs = x @ c
  e      = where(adj, e, -9e15);  att = softmax(e, axis=1);  out = att @ h

Device strategy (1D row-parallel, transposed score layout):
  - Each core owns a 1024-row block of the score matrix; score tiles are
    computed TRANSPOSED: [j on partitions, i on free dim] so the final
    contraction over j maps directly onto the tensor engine
    (lhsT = p subtile [j, 128i], rhs = [h | 1 | 1] tile [j, 258]) and the
    softmax denominator falls out of the ones columns for free.

  - exp() is eliminated entirely (the baseline spent ~55us/core of
    ScalarE on it): all logits are computed pre-scaled by
    K = 1024*log2(e), so the integer i16 = K*e + B IS the fp16 bit
    pattern of ~exp(e-4).  The second DVE pass emits i16 as uint16
    directly and the matmul reads the same bytes as fp16.  B includes a
    -58.68 Schraudolph centering for the mantissa-linear interpolation
    (~1.8% rms, zero-mean in log space; cancels further in the softmax
    ratio).  Masked entries (NaN in the fp16 causal-weight matrix) flow
    NaN -> MAX(,0) -> +0.0, i.e. the mask costs nothing.

  - Elementwise work is two 2x-packed custom DVE passes per j-tile
    (DVE is the bottleneck engine at ~0.96 GHz, 2 fp16 elem/cycle/lane):
      W3:    w   = fmax((negcs_i + K*cs_j) * M_ij, -50000)          [3 ops]
      LADD4: p16 = uint16( max( max(ss2_i + c1_j, c2_j) + w, 0 ) )  [4 ops]
    with negcs = -K*cs, ss2 = -0.8*K*ss per-i fp16 streams and
    c1 = 0.2*K*sd_j + B, c2 = K*sd_j + B, K*cs_j per-partition scalars.
    The 4-op LADD4 packs into all 8 DVE slices (stages 0-3 lo, 4-7 hi).
    All W3s of a quad are issued before its LADD4s so the dependent op
    never waits on the producing op's write acknowledgement.

  - h is shipped from the host (x@W is 0.8% of the kernel FLOPs) as fp16
    [N, 258] with the ones columns baked in: no on-device h pass, no
    PSUM->SBUF copies, no h DMA dependency on the critical path.  The
    per-i/per-j score projections (cs = x@c, ss = x@(W@a1), sd = x@(W@a2),
    O(N*F)) are also host-side, so the DVE pipeline starts as soon as the
    first causal-weight chunk lands (~2us).
"""

import numpy as np

import concourse.mybir as mybir
import concourse.tile as tile
from concourse import bacc
from concourse import dve_ops as _dops
from concourse.bass_utils import run_bass_kernel_spmd
from concourse.dve_ops import DveOp, get_dve_sub_opcode
from concourse.dve_spec import (C0, C1, Spec, Src0, Src1, Zero, _has_src1,
                                lower, maxx)
from concourse.dve_uop import (ENABLE, AluInp, AluOp, DelayInp, DveOpSpec,
                               InpSel, OutPath, OutSel, Trigger, UopConfig,
                               UopDpConfig)

dt = mybir.dt
AF = mybir.ActivationFunctionType

N = 8192
F = 256
NCORES = 8
RPC = N // NCORES          # rows per core (i range)
NJT = N // 128             # j tiles of 128
NSUB = RPC // 128          # i subtiles of 128
NMM = F + 2                # matmul rhs width: [h | 1 | 1]
MASK_NEG = -50000.0        # masked w: guarantees max(.,0) clamps to +0

K_SCALE = 1024.0 * np.log2(np.e)            # 1477.3197
# bit-trick bias: fp16 exponent bias (15<<10) - 4*K (the exp(-4) shift)
# - 58.68 (Schraudolph centering of the mantissa-linear 2^x)
B_BIAS = 15360.0 - 4.0 * K_SCALE - 58.68

_PAL = AluInp.PREV_ALU_OUT
_D = (AluInp.PREV_DELAY_0, AluInp.PREV_DELAY_1, AluInp.PREV_DELAY_2,
      AluInp.PREV_DELAY_3, AluInp.PREV_DELAY_4, AluInp.PREV_DELAY_5)


def _w3_2x_uop():
    """w = max((Src1 + C0) * Src0, C1); 3-op body packed 2x (6 stages)."""
    u = UopConfig()
    for sel, slot in [(InpSel.SRC_1, 0), (InpSel.CONST_0, 1), (InpSel.SRC_0, 2),
                      (InpSel.CONST_1, 3), (InpSel.SRC_1_HI, 4),
                      (InpSel.SRC_0_HI, 5)]:
        u.enable_input(sel, slot)
    lanes = (0, 1, 2, 3, 4)
    dp = [UopDpConfig() for _ in range(8)]
    stages = [
        (AluOp.ADD, _PAL, _D[0]),        # negcs_lo + cs_j
        (AluOp.MULTIPLY, _PAL, _D[1]),   # * M_lo
        (AluOp.MAX, _PAL, _D[2]),        # max(, -50000)
        (AluOp.ADD, _D[3], _D[0]),       # negcs_hi + cs_j   (+ lo capture)
        (AluOp.MULTIPLY, _PAL, _D[4]),   # * M_hi
        (AluOp.MAX, _PAL, _D[2]),        # max(, -50000)
    ]
    for st, (op, a, b) in enumerate(stages):
        dp[st].enable_alu(op, a, b)
        dp[st].pass_through_delay(*(lanes if st < 4 else (*lanes, 5)))
    dp[3].enable_delay_from_src(DelayInp.PREV_ALU_OUT, 5)
    for st in range(6, 8):
        dp[st].enable_alu(AluOp.BYPASS, AluInp.PREV_ALU_OUT, AluInp.PREV_ALU_OUT)
        dp[st].pass_through_delay(*lanes, 5)
    u.datapath_config = dp
    u.enable_output(OutSel.DELAY_5, OutPath.WR0_LO)
    u.enable_output(OutSel.ALU_OUT, OutPath.WR0_HI)
    u.require_inp0 = ENABLE
    u.require_inp1 = ENABLE
    u.trigger = (Trigger.SRC_TENSOR_DONE, Trigger.NONE, Trigger.NONE)
    u.next_uop = (0, 0, 0)
    return u


def _ladd4_2x_uop():
    """p = max(max(ss2 + C0, C1) + w, 0); 4-op body packed 2x (all 8 stages).

    input slots: 0: SRC_0 (ss2 lo -> ALU lane), 1: CONST_0 (c1) -> d0,
      2: CONST_1 (c2) -> d1, 3: SRC_1 (w lo) -> d2, 4: ZERO -> d3,
      5: SRC_0_HI (ss2 hi) -> d4, 6: SRC_1_HI (w hi) -> d5.
    lo runs stages 0-3; stage 4 captures the lo result into d2 (w_lo is
    dead there) while starting the hi half on stages 4-7."""
    u = UopConfig()
    u.enable_input(InpSel.SRC_0, 0)
    u.enable_input(InpSel.CONST_0, 1)
    u.enable_input(InpSel.CONST_1, 2)
    u.enable_input(InpSel.SRC_1, 3)
    u.enable_input(InpSel.ZERO, 4)
    u.enable_input(InpSel.SRC_0_HI, 5)
    u.enable_input(InpSel.SRC_1_HI, 6)
    dp = [UopDpConfig() for _ in range(8)]
    dp[0].enable_alu(AluOp.ADD, _PAL, _D[0]).pass_through_delay(0, 1, 2, 3, 4, 5)
    dp[1].enable_alu(AluOp.MAX, _PAL, _D[1]).pass_through_delay(0, 1, 2, 3, 4, 5)
    dp[2].enable_alu(AluOp.ADD, _PAL, _D[2]).pass_through_delay(0, 1, 3, 4, 5)
    dp[3].enable_alu(AluOp.MAX, _PAL, _D[3]).pass_through_delay(0, 1, 3, 4, 5)
    dp[4].enable_alu(AluOp.ADD, _D[4], _D[0])
    dp[4].enable_delay_from_src(DelayInp.PREV_ALU_OUT, 2)
    dp[4].pass_through_delay(1, 3, 5)
    dp[5].enable_alu(AluOp.MAX, _PAL, _D[1]).pass_through_delay(2, 3, 5)
    dp[6].enable_alu(AluOp.ADD, _PAL, _D[5]).pass_through_delay(2, 3)
    dp[7].enable_alu(AluOp.MAX, _PAL, _D[3]).pass_through_delay(2)
    u.datapath_config = dp
    u.enable_output(OutSel.DELAY_2, OutPath.WR0_LO)
    u.enable_output(OutSel.ALU_OUT, OutPath.WR0_HI)
    u.require_inp0 = ENABLE
    u.require_inp1 = ENABLE
    u.trigger = (Trigger.SRC_TENSOR_DONE, Trigger.NONE, Trigger.NONE)
    u.next_uop = (0, 0, 0)
    return u


_UOP2X_BUILDERS = {"CGA_W3": _w3_2x_uop, "CGA_LADD4": _ladd4_2x_uop}


class DveOp2x(DveOp):
    """DveOp whose compiled table also carries a hand-written 2x_1p uop
    program; emitted instructions additionally set perf_max=1 so the
    engine may select the packed mode when dtype/stride conditions hold."""

    def compile(self, ver):
        key = ("2x:" + self.name, ver)
        cached = _dops._COMPILE_CACHE.get(key)
        if cached is not None:
            return cached
        result = DveOpSpec(
            name=self.name,
            opcode=get_dve_sub_opcode(self.name),
            uops=lower(self.spec, ver=ver),
            rd1_en=_has_src1(self.spec),
            uops_2x=[_UOP2X_BUILDERS[self.name]()] if ver == "v3" else None,
            perf_max=1 if ver == "v3" else 0,
        )
        _dops._COMPILE_CACHE[key] = result
        return result


def _register(name, spec):
    for op in _dops.OPS:
        if op.name == name:
            return op
    opcode = _dops._CUSTOM_DVE_ROW_BASE + len(_dops.OPS)
    assert opcode < 0x20
    _dops._SUB_OPCODE_FOR_NAME[name] = opcode
    shas = {}
    for ver in ("v3", "v4"):
        s = DveOpSpec(name=name, opcode=opcode, uops=lower(spec, ver=ver),
                      rd1_en=_has_src1(spec))
        shas[ver] = s.sha(ver)
    op = DveOp2x(name, spec, subdim=False, uops_sha=shas)
    _dops.OPS.append(op)
    _dops.CUSTOM_DVE_SPECS[name] = op.spec
    return op


# w = max((negcs + K*cs_j) * M, -50000): causal product + NaN-encoded mask
W3_OP = _register("CGA_W3", Spec(
    body=maxx((Src1 + C0) * Src0, C1),
    reference=lambda in0, in1, s0, s1: np.fmax((in1 + s0) * in0, s1)))

# p16 = max(max(ss2 + c1_j, c2_j) + w, 0): leaky factor + exp bit trick
LADD4_OP = _register("CGA_LADD4", Spec(
    body=maxx(maxx(Src0 + C0, C1) + Src1, Zero),
    reference=lambda in0, in1, s0, s1: np.fmax(np.maximum(in0 + s0, s1) + in1,
                                               0.0)))


def _emit2x(nc, op, out, in0, in1, s0, s1):
    bi = nc.vector._custom_dve(op, out=out, in0=in0, in1=in1, s0=s0, s1=s1)
    bi.ins.perf_max = 1
    return bi


def build_program():
    nc = bacc.Bacc("TRN2", target_bir_lowering=False, debug=False,
                   num_devices=NCORES)

    h16_d = nc.declare_dram_parameter("h16", [N, NMM], dt.float16, isOutput=False)
    cwmT = nc.declare_dram_parameter("cwmT", [N, RPC], dt.float16, isOutput=False)
    negcs_d = nc.declare_dram_parameter("negcs", [128, RPC], dt.float16, isOutput=False)
    ss2_d = nc.declare_dram_parameter("ss2", [128, RPC], dt.float16, isOutput=False)
    scols_d = nc.declare_dram_parameter("scols", [128, NJT, 3], dt.float32, isOutput=False)
    out_d = nc.declare_dram_parameter("out", [RPC, F], dt.float32, isOutput=True)
    dbg_w_d = nc.declare_dram_parameter("dbg_w", [128, 128], dt.float16, isOutput=True)
    dbg_p_d = nc.declare_dram_parameter("dbg_p", [128, 128], dt.uint16, isOutput=True)

    with tile.TileContext(nc) as tc:
        with (
            tc.tile_pool(name="persist", bufs=1) as persist,
            tc.tile_pool(name="main", bufs=2) as main_pool,
            tc.tile_pool(name="tail", bufs=2) as tailp,
        ):
            # --- persistent tiles ---
            h_sb = persist.tile([128, NJT, NMM], dt.float16, tag="h16")
            negcs_repl = persist.tile([128, RPC], dt.float16, tag="negcs")
            ss2_repl = persist.tile([128, RPC], dt.float16, tag="ss2")
            scols = persist.tile([128, NJT, 3], dt.float32, tag="scols")
            dbg_w = persist.tile([128, 128], dt.float16, tag="dbg_w")
            dbg_p = persist.tile([128, 128], dt.uint16, tag="dbg_p")

            # small tensors first: they gate the DVE pipeline start
            nc.sync.dma_start(out=negcs_repl[:], in_=negcs_d.ap())
            nc.sync.dma_start(out=ss2_repl[:], in_=ss2_d.ap())
            nc.sync.dma_start(out=scols[:], in_=scols_d.ap())

            cw_src = cwmT.ap().rearrange("(c p) i -> p c i", p=128)
            cw_tiles = {}

            def fetch_cw(jq):
                if jq not in cw_tiles and jq < NJT // 4:
                    t = main_pool.tile([128, 4, RPC], dt.float16, tag="cw",
                                       bufs=4, name=f"cw{jq}")
                    nc.sync.dma_start(out=t[:], in_=cw_src[:, 4 * jq:4 * jq + 4, :])
                    cw_tiles[jq] = t
                return cw_tiles.get(jq)

            fetch_cw(0)
            fetch_cw(1)

            # h (with baked ones columns), in 8 chunks; first chunks gate the
            # earliest matmuls
            h_src = h16_d.ap().rearrange("(t p) c -> p t c", p=128)
            for blk in range(8):
                sl = slice(blk * (NJT // 8), (blk + 1) * (NJT // 8))
                nc.sync.dma_start(out=h_sb[:, sl, :], in_=h_src[:, sl, :])

            # --- main loop: 4 j-tiles (1 cw quad) per iteration ---
            with tc.tile_pool(name="psum_o", bufs=1, space="PSUM") as psum_o:
                out_ps = [psum_o.tile([128, NMM], dt.float32, tag=f"out{s}",
                                      name=f"out_ps{s}")
                          for s in range(NSUB)]

                for jq in range(NJT // 4):
                    w_quad = main_pool.tile([128, 4, RPC], dt.float16, tag="w", bufs=2)
                    p_quad = main_pool.tile([128, 4, RPC], dt.uint16, tag="p", bufs=3)
                    cw_t = fetch_cw(jq)
                    fetch_cw(jq + 1)
                    fetch_cw(jq + 2)
                    for q in range(4):
                        jt = 4 * jq + q
                        _emit2x(nc, W3_OP, out=w_quad[:, q, :], in0=cw_t[:, q, :],
                                in1=negcs_repl[:], s0=scols[:, jt, 2:3],
                                s1=MASK_NEG)
                    for q in range(4):
                        jt = 4 * jq + q
                        _emit2x(nc, LADD4_OP, out=p_quad[:, q, :], in0=ss2_repl[:],
                                in1=w_quad[:, q, :], s0=scols[:, jt, 1:2],
                                s1=scols[:, jt, 0:1])
                    # out[i, :] += p.T @ [h | 1]
                    p16v = p_quad[:].bitcast(dt.float16)
                    for q in range(4):
                        jt = 4 * jq + q
                        for s in range(NSUB):
                            nc.tensor.matmul(
                                out_ps[s][:],
                                lhsT=p16v[:, q, s * 128:(s + 1) * 128],
                                rhs=h_sb[:, jt, :],
                                start=(jt == 0), stop=(jt == NJT - 1))

                # on-device selftest of both packed 2x ops (jt=0 inputs)
                _emit2x(nc, W3_OP, out=dbg_w[:], in0=cw_tiles[0][:, 0, 0:128],
                        in1=negcs_repl[:, 0:128], s0=scols[:, 0, 2:3],
                        s1=MASK_NEG)
                _emit2x(nc, LADD4_OP, out=dbg_p[:], in0=ss2_repl[:, 0:128],
                        in1=dbg_w[:], s0=scols[:, 0, 1:2], s1=scols[:, 0, 0:1])
                nc.sync.dma_start(out=dbg_w_d.ap(), in_=dbg_w[:])
                nc.sync.dma_start(out=dbg_p_d.ap(), in_=dbg_p[:])

                # --- tail: normalize and write out ---
                o_all = tailp.tile([128, NSUB, F], dt.float32, tag="osb", bufs=1)
                for s in range(NSUB):
                    rec = tailp.tile([128, 1], dt.float32, tag="rec", bufs=4)
                    nc.vector.reciprocal(rec[:], out_ps[s][:, F:F + 1])
                    nc.scalar.activation(o_all[:, s, :], out_ps[s][:, 0:F], AF.Copy,
                                         scale=rec[:])
                nc.sync.dma_start(out=out_d.ap().rearrange("(s p) f -> p s f", p=128),
                                  in_=o_all[:])

    nc.compile()
    return nc


_CACHED_NC = None


def _get_program():
    global _CACHED_NC
    if _CACHED_NC is None:
        _CACHED_NC = build_program()
    return _CACHED_NC


def _host_prep(x, adj, causal_weights, W, a1, a2, c):
    x = np.asarray(x, dtype=np.float32)
    adj = np.asarray(adj)
    cw = np.asarray(causal_weights, dtype=np.float32)
    W = np.asarray(W, dtype=np.float32)
    a1 = np.asarray(a1, dtype=np.float32)
    a2 = np.asarray(a2, dtype=np.float32)
    c = np.asarray(c, dtype=np.float32)

    # projections + h on host (O(N*F) / 0.8% of kernel FLOPs)
    cs = x @ c                      # [N]
    ss = x @ (W @ a1)               # [N]
    sd = x @ (W @ a2)               # [N]
    h16 = np.ones((N, NMM), dtype=np.float16)
    h16[:, 0:F] = (x @ W).astype(np.float16)

    # per-j scalar columns [128, NJT, 3]: [K*sd+B, 0.2*K*sd+B, K*cs], j = t*128+p
    sd_t = sd.reshape(NJT, 128).T   # [128, NJT]
    cs_t = cs.reshape(NJT, 128).T
    scols = np.stack([K_SCALE * sd_t + B_BIAS,
                      0.2 * K_SCALE * sd_t + B_BIAS,
                      K_SCALE * cs_t], axis=2).astype(np.float32)

    # NaN-encoded mask: edge -> causal weight, non-edge -> NaN (the DVE MAX
    # suppresses NaN; LADD4's final MAX(,0) maps any masked residue to +0)
    cwm = np.where(adj > 0, cw, np.nan).astype(np.float16)

    in_maps = []
    for k in range(NCORES):
        r0, r1 = k * RPC, (k + 1) * RPC
        negcs = np.repeat((-K_SCALE * cs[r0:r1]).astype(np.float16)[None, :],
                          128, axis=0)
        ss2 = np.repeat((-0.8 * K_SCALE * ss[r0:r1]).astype(np.float16)[None, :],
                        128, axis=0)
        in_maps.append({
            "h16": h16,
            "cwmT": np.ascontiguousarray(cwm[r0:r1, :].T),
            "negcs": negcs,
            "ss2": ss2,
            "scols": scols,
        })
    return in_maps


def kernel(x, adj, causal_weights, W, a1, a2, c, _trace=False, _trace_kwargs=None):
    nc = _get_program()
    in_maps = _host_prep(x, adj, causal_weights, W, a1, a2, c)
    kw = {}
    if _trace:
        kw["trace"] = True
        kw.update(_trace_kwargs or {})
    res = run_bass_kernel_spmd(nc, in_maps, list(range(NCORES)), **kw)
    out = np.concatenate([res.results[k]["out"] for k in range(NCORES)], axis=0)

    # host check of the on-device 2x custom-op self-tests (core 0, jt=0)
    try:
        m0 = in_maps[0]
        negf = m0["negcs"][:, 0:128].astype(np.float32)
        cs_j = m0["scols"][:, 0, 2:3]
        cwm0 = m0["cwmT"][0:128, 0:128].astype(np.float32)
        w_exp = np.fmax((negf + cs_j) * cwm0, MASK_NEG)
        w_got = res.results[0]["dbg_w"].astype(np.float32)
        dw = np.abs(w_got - w_exp)
        print(f"[w3-2x selftest] max abs diff: {np.nanmax(dw):.4f}")
        ss2f = m0["ss2"][:, 0:128].astype(np.float32)
        i_exp = np.fmax(np.maximum(ss2f + m0["scols"][:, 0, 1:2],
                                   m0["scols"][:, 0, 0:1]) + w_got, 0.0)
        p_got = res.results[0]["dbg_p"].astype(np.int64)
        dp_ = np.abs(p_got - i_exp.astype(np.int64))
        print(f"[ladd4-2x selftest] max |diff|: {dp_.max()}")
    except Exception as exc:  # pragma: no cover - diagnostic only
        print(f"[2x selftest] check failed: {exc}")

    if _trace:
        return out, res
    return out


# revision 8
# speedup vs baseline: 1.1220x; 1.1195x over previous
"""Trainium2 Bass kernel for CausalGraphAttention (N=8192, F=256), 8-core SPMD.

Math (per reference):
  h      = x @ W                               [N, F]
  e[i,j] = leaky_relu(h[i]@a1 + h[j]@a2, 0.2)
           + (cs[j] - cs[i]) * cw[i,j],   cs = x @ c
  e      = where(adj, e, -9e15);  att = softmax(e, axis=1);  out = att @ h

Device strategy (1D row-parallel, transposed score layout):
  - Each core owns a 1024-row block of the score matrix; score tiles are
    computed TRANSPOSED: [j on partitions, i on free dim] so the final
    contraction over j maps directly onto the tensor engine
    (lhsT = p subtile [j, 128i], rhs = [h | 1 | 1] tile [j, 258]) and the
    softmax denominator falls out of the ones columns for free.

  - exp() is eliminated entirely (the baseline spent ~55us/core of
    ScalarE on it): all logits are computed pre-scaled by
    K = 1024*log2(e), so the integer i16 = K*e + B IS the fp16 bit
    pattern of ~exp(e-4).  The second DVE pass emits i16 as uint16
    directly and the matmul reads the same bytes as fp16.  B includes a
    -58.68 Schraudolph centering for the mantissa-linear interpolation
    (~1.8% rms, zero-mean in log space; cancels further in the softmax
    ratio).  Masked entries (NaN in the fp16 causal-weight matrix) flow
    NaN -> MAX(,0) -> +0.0, i.e. the mask costs nothing.

  - Elementwise work is two 2x-packed custom DVE passes per j-tile
    (DVE is the bottleneck engine at ~0.96 GHz, 2 fp16 elem/cycle/lane):
      W3:    w   = fmax((negcs_i + K*cs_j) * M_ij, -50000)          [3 ops]
      LADD4: p16 = uint16( max( max(ss2_i + c1_j, c2_j) + w, 0 ) )  [4 ops]
    with negcs = -K*cs, ss2 = -0.8*K*ss per-i fp16 streams and
    c1 = 0.2*K*sd_j + B, c2 = K*sd_j + B, K*cs_j per-partition scalars.
    The 4-op LADD4 packs into all 8 DVE slices (stages 0-3 lo, 4-7 hi).
    All W3s of a quad are issued before its LADD4s so the dependent op
    never waits on the producing op's write acknowledgement.

  - h is shipped from the host (x@W is 0.8% of the kernel FLOPs) as fp16
    [N, 258] with the ones columns baked in: no on-device h pass, no
    PSUM->SBUF copies, no h DMA dependency on the critical path.  The
    per-i/per-j score projections (cs = x@c, ss = x@(W@a1), sd = x@(W@a2),
    O(N*F)) are also host-side, so the DVE pipeline starts as soon as the
    first causal-weight chunk lands (~2us).
"""

import numpy as np

import concourse.mybir as mybir
import concourse.tile as tile
from concourse import bacc
from concourse import dve_ops as _dops
from concourse.bass_utils import run_bass_kernel_spmd
from concourse.dve_ops import DveOp, get_dve_sub_opcode
from concourse.dve_spec import (C0, C1, Spec, Src0, Src1, Zero, _has_src1,
                                lower, maxx)
from concourse.dve_uop import (ENABLE, AluInp, AluOp, DelayInp, DveOpSpec,
                               InpSel, OutPath, OutSel, Trigger, UopConfig,
                               UopDpConfig)

dt = mybir.dt
AF = mybir.ActivationFunctionType

N = 8192
F = 256
NCORES = 8
RPC = N // NCORES          # rows per core (i range)
NJT = N // 128             # j tiles of 128
NSUB = RPC // 128          # i subtiles of 128
NMM = F + 2                # matmul rhs width: [h | 1 | 1]
MASK_NEG = -50000.0        # masked w: guarantees max(.,0) clamps to +0

K_SCALE = 1024.0 * np.log2(np.e)            # 1477.3197
# bit-trick bias: fp16 exponent bias (15<<10) - 4*K (the exp(-4) shift)
# - 58.68 (Schraudolph centering of the mantissa-linear 2^x)
B_BIAS = 15360.0 - 4.0 * K_SCALE - 58.68

_PAL = AluInp.PREV_ALU_OUT
_D = (AluInp.PREV_DELAY_0, AluInp.PREV_DELAY_1, AluInp.PREV_DELAY_2,
      AluInp.PREV_DELAY_3, AluInp.PREV_DELAY_4, AluInp.PREV_DELAY_5)


def _w3_2x_uop():
    """w = max((Src1 + C0) * Src0, C1); 3-op body packed 2x (6 stages)."""
    u = UopConfig()
    for sel, slot in [(InpSel.SRC_1, 0), (InpSel.CONST_0, 1), (InpSel.SRC_0, 2),
                      (InpSel.CONST_1, 3), (InpSel.SRC_1_HI, 4),
                      (InpSel.SRC_0_HI, 5)]:
        u.enable_input(sel, slot)
    lanes = (0, 1, 2, 3, 4)
    dp = [UopDpConfig() for _ in range(8)]
    stages = [
        (AluOp.ADD, _PAL, _D[0]),        # negcs_lo + cs_j
        (AluOp.MULTIPLY, _PAL, _D[1]),   # * M_lo
        (AluOp.MAX, _PAL, _D[2]),        # max(, -50000)
        (AluOp.ADD, _D[3], _D[0]),       # negcs_hi + cs_j   (+ lo capture)
        (AluOp.MULTIPLY, _PAL, _D[4]),   # * M_hi
        (AluOp.MAX, _PAL, _D[2]),        # max(, -50000)
    ]
    for st, (op, a, b) in enumerate(stages):
        dp[st].enable_alu(op, a, b)
        dp[st].pass_through_delay(*(lanes if st < 4 else (*lanes, 5)))
    dp[3].enable_delay_from_src(DelayInp.PREV_ALU_OUT, 5)
    for st in range(6, 8):
        dp[st].enable_alu(AluOp.BYPASS, AluInp.PREV_ALU_OUT, AluInp.PREV_ALU_OUT)
        dp[st].pass_through_delay(*lanes, 5)
    u.datapath_config = dp
    u.enable_output(OutSel.DELAY_5, OutPath.WR0_LO)
    u.enable_output(OutSel.ALU_OUT, OutPath.WR0_HI)
    u.require_inp0 = ENABLE
    u.require_inp1 = ENABLE
    u.trigger = (Trigger.SRC_TENSOR_DONE, Trigger.NONE, Trigger.NONE)
    u.next_uop = (0, 0, 0)
    return u


def _ladd4_2x_uop():
    """p = max(max(ss2 + C0, C1) + w, 0); 4-op body packed 2x (all 8 stages).

    input slots: 0: SRC_0 (ss2 lo -> ALU lane), 1: CONST_0 (c1) -> d0,
      2: CONST_1 (c2) -> d1, 3: SRC_1 (w lo) -> d2, 4: ZERO -> d3,
      5: SRC_0_HI (ss2 hi) -> d4, 6: SRC_1_HI (w hi) -> d5.
    lo runs stages 0-3; stage 4 captures the lo result into d2 (w_lo is
    dead there) while starting the hi half on stages 4-7."""
    u = UopConfig()
    u.enable_input(InpSel.SRC_0, 0)
    u.enable_input(InpSel.CONST_0, 1)
    u.enable_input(InpSel.CONST_1, 2)
    u.enable_input(InpSel.SRC_1, 3)
    u.enable_input(InpSel.ZERO, 4)
    u.enable_input(InpSel.SRC_0_HI, 5)
    u.enable_input(InpSel.SRC_1_HI, 6)
    dp = [UopDpConfig() for _ in range(8)]
    dp[0].enable_alu(AluOp.ADD, _PAL, _D[0]).pass_through_delay(0, 1, 2, 3, 4, 5)
    dp[1].enable_alu(AluOp.MAX, _PAL, _D[1]).pass_through_delay(0, 1, 2, 3, 4, 5)
    dp[2].enable_alu(AluOp.ADD, _PAL, _D[2]).pass_through_delay(0, 1, 3, 4, 5)
    dp[3].enable_alu(AluOp.MAX, _PAL, _D[3]).pass_through_delay(0, 1, 3, 4, 5)
    dp[4].enable_alu(AluOp.ADD, _D[4], _D[0])
    dp[4].enable_delay_from_src(DelayInp.PREV_ALU_OUT, 2)
    dp[4].pass_through_delay(1, 3, 5)
    dp[5].enable_alu(AluOp.MAX, _PAL, _D[1]).pass_through_delay(2, 3, 5)
    dp[6].enable_alu(AluOp.ADD, _PAL, _D[5]).pass_through_delay(2, 3)
    dp[7].enable_alu(AluOp.MAX, _PAL, _D[3]).pass_through_delay(2)
    u.datapath_config = dp
    u.enable_output(OutSel.DELAY_2, OutPath.WR0_LO)
    u.enable_output(OutSel.ALU_OUT, OutPath.WR0_HI)
    u.require_inp0 = ENABLE
    u.require_inp1 = ENABLE
    u.trigger = (Trigger.SRC_TENSOR_DONE, Trigger.NONE, Trigger.NONE)
    u.next_uop = (0, 0, 0)
    return u


_UOP2X_BUILDERS = {"CGA_W3": _w3_2x_uop, "CGA_LADD4": _ladd4_2x_uop}


class DveOp2x(DveOp):
    """DveOp whose compiled table also carries a hand-written 2x_1p uop
    program; emitted instructions additionally set perf_max=1 so the
    engine may select the packed mode when dtype/stride conditions hold."""

    def compile(self, ver):
        key = ("2x:" + self.name, ver)
        cached = _dops._COMPILE_CACHE.get(key)
        if cached is not None:
            return cached
        result = DveOpSpec(
            name=self.name,
            opcode=get_dve_sub_opcode(self.name),
            uops=lower(self.spec, ver=ver),
            rd1_en=_has_src1(self.spec),
            uops_2x=[_UOP2X_BUILDERS[self.name]()] if ver == "v3" else None,
            perf_max=1 if ver == "v3" else 0,
        )
        _dops._COMPILE_CACHE[key] = result
        return result


def _register(name, spec):
    for op in _dops.OPS:
        if op.name == name:
            return op
    opcode = _dops._CUSTOM_DVE_ROW_BASE + len(_dops.OPS)
    assert opcode < 0x20
    _dops._SUB_OPCODE_FOR_NAME[name] = opcode
    shas = {}
    for ver in ("v3", "v4"):
        s = DveOpSpec(name=name, opcode=opcode, uops=lower(spec, ver=ver),
                      rd1_en=_has_src1(spec))
        shas[ver] = s.sha(ver)
    op = DveOp2x(name, spec, subdim=False, uops_sha=shas)
    _dops.OPS.append(op)
    _dops.CUSTOM_DVE_SPECS[name] = op.spec
    return op


# w = max((negcs + K*cs_j) * M, -50000): causal product + NaN-encoded mask
W3_OP = _register("CGA_W3", Spec(
    body=maxx((Src1 + C0) * Src0, C1),
    reference=lambda in0, in1, s0, s1: np.fmax((in1 + s0) * in0, s1)))

# p16 = max(max(ss2 + c1_j, c2_j) + w, 0): leaky factor + exp bit trick
LADD4_OP = _register("CGA_LADD4", Spec(
    body=maxx(maxx(Src0 + C0, C1) + Src1, Zero),
    reference=lambda in0, in1, s0, s1: np.fmax(np.maximum(in0 + s0, s1) + in1,
                                               0.0)))


def _emit2x(nc, op, out, in0, in1, s0, s1):
    bi = nc.vector._custom_dve(op, out=out, in0=in0, in1=in1, s0=s0, s1=s1)
    bi.ins.perf_max = 1
    return bi


def build_program():
    nc = bacc.Bacc("TRN2", target_bir_lowering=False, debug=False,
                   num_devices=NCORES)

    h16_d = nc.declare_dram_parameter("h16", [N, NMM], dt.float16, isOutput=False)
    cwmT = nc.declare_dram_parameter("cwmT", [N, RPC], dt.float16, isOutput=False)
    negcs_d = nc.declare_dram_parameter("negcs", [128, RPC], dt.float16, isOutput=False)
    ss2_d = nc.declare_dram_parameter("ss2", [128, RPC], dt.float16, isOutput=False)
    scols_d = nc.declare_dram_parameter("scols", [128, NJT, 3], dt.float32, isOutput=False)
    out_d = nc.declare_dram_parameter("out", [RPC, F], dt.float32, isOutput=True)
    dbg_w_d = nc.declare_dram_parameter("dbg_w", [128, 128], dt.float16, isOutput=True)
    dbg_p_d = nc.declare_dram_parameter("dbg_p", [128, 128], dt.uint16, isOutput=True)

    with tile.TileContext(nc) as tc:
        with (
            tc.tile_pool(name="persist", bufs=1) as persist,
            tc.tile_pool(name="main", bufs=2) as main_pool,
            tc.tile_pool(name="tail", bufs=2) as tailp,
        ):
            # --- persistent tiles ---
            h_sb = persist.tile([128, NJT, NMM], dt.float16, tag="h16")
            negcs_repl = persist.tile([128, RPC], dt.float16, tag="negcs")
            ss2_repl = persist.tile([128, RPC], dt.float16, tag="ss2")
            scols = persist.tile([128, NJT, 3], dt.float32, tag="scols")
            dbg_w = persist.tile([128, 128], dt.float16, tag="dbg_w")
            dbg_p = persist.tile([128, 128], dt.uint16, tag="dbg_p")

            # small tensors first: they gate the DVE pipeline start
            nc.sync.dma_start(out=negcs_repl[:], in_=negcs_d.ap())
            nc.sync.dma_start(out=ss2_repl[:], in_=ss2_d.ap())
            nc.sync.dma_start(out=scols[:], in_=scols_d.ap())

            cw_src = cwmT.ap().rearrange("(c p) i -> p c i", p=128)
            cw_tiles = {}

            def fetch_cw(jq):
                if jq not in cw_tiles and jq < NJT // 4:
                    t = main_pool.tile([128, 4, RPC], dt.float16, tag="cw",
                                       bufs=6, name=f"cw{jq}")
                    nc.sync.dma_start(out=t[:], in_=cw_src[:, 4 * jq:4 * jq + 4, :])
                    cw_tiles[jq] = t
                return cw_tiles.get(jq)

            fetch_cw(0)
            fetch_cw(1)

            # h (with baked ones columns) on the ScalarE DMA queue: the sync
            # queue stays a pure causal-weight stream (no head blocking)
            h_src = h16_d.ap().rearrange("(t p) c -> p t c", p=128)
            for blk in range(8):
                sl = slice(blk * (NJT // 8), (blk + 1) * (NJT // 8))
                nc.scalar.dma_start(out=h_sb[:, sl, :], in_=h_src[:, sl, :])

            # --- main loop: 4 j-tiles (1 cw quad) per iteration ---
            with tc.tile_pool(name="psum_o", bufs=1, space="PSUM") as psum_o:
                out_ps = [psum_o.tile([128, NMM], dt.float32, tag=f"out{s}",
                                      name=f"out_ps{s}")
                          for s in range(NSUB)]

                for jq in range(NJT // 4):
                    w_quad = main_pool.tile([128, 4, RPC], dt.float16, tag="w", bufs=2)
                    p_quad = main_pool.tile([128, 4, RPC], dt.uint16, tag="p", bufs=3)
                    cw_t = fetch_cw(jq)
                    fetch_cw(jq + 1)
                    fetch_cw(jq + 2)
                    fetch_cw(jq + 3)
                    for q in range(4):
                        jt = 4 * jq + q
                        _emit2x(nc, W3_OP, out=w_quad[:, q, :], in0=cw_t[:, q, :],
                                in1=negcs_repl[:], s0=scols[:, jt, 2:3],
                                s1=MASK_NEG)
                    for q in range(4):
                        jt = 4 * jq + q
                        _emit2x(nc, LADD4_OP, out=p_quad[:, q, :], in0=ss2_repl[:],
                                in1=w_quad[:, q, :], s0=scols[:, jt, 1:2],
                                s1=scols[:, jt, 0:1])
                    # out[i, :] += p.T @ [h | 1]
                    p16v = p_quad[:].bitcast(dt.float16)
                    for q in range(4):
                        jt = 4 * jq + q
                        for s in range(NSUB):
                            nc.tensor.matmul(
                                out_ps[s][:],
                                lhsT=p16v[:, q, s * 128:(s + 1) * 128],
                                rhs=h_sb[:, jt, :],
                                start=(jt == 0), stop=(jt == NJT - 1))

                # on-device selftest of both packed 2x ops (jt=0 inputs)
                _emit2x(nc, W3_OP, out=dbg_w[:], in0=cw_tiles[0][:, 0, 0:128],
                        in1=negcs_repl[:, 0:128], s0=scols[:, 0, 2:3],
                        s1=MASK_NEG)
                _emit2x(nc, LADD4_OP, out=dbg_p[:], in0=ss2_repl[:, 0:128],
                        in1=dbg_w[:], s0=scols[:, 0, 1:2], s1=scols[:, 0, 0:1])
                nc.scalar.dma_start(out=dbg_w_d.ap(), in_=dbg_w[:])
                nc.scalar.dma_start(out=dbg_p_d.ap(), in_=dbg_p[:])

                # --- tail: normalize and write out ---
                o_all = tailp.tile([128, NSUB, F], dt.float32, tag="osb", bufs=1)
                for s in range(NSUB):
                    rec = tailp.tile([128, 1], dt.float32, tag="rec", bufs=4)
                    nc.vector.reciprocal(rec[:], out_ps[s][:, F:F + 1])
                    nc.scalar.activation(o_all[:, s, :], out_ps[s][:, 0:F], AF.Copy,
                                         scale=rec[:])
                nc.scalar.dma_start(out=out_d.ap().rearrange("(s p) f -> p s f", p=128),
                                    in_=o_all[:])

    nc.compile()
    return nc


_CACHED_NC = None


def _get_program():
    global _CACHED_NC
    if _CACHED_NC is None:
        _CACHED_NC = build_program()
    return _CACHED_NC


def _host_prep(x, adj, causal_weights, W, a1, a2, c):
    x = np.asarray(x, dtype=np.float32)
    adj = np.asarray(adj)
    cw = np.asarray(causal_weights, dtype=np.float32)
    W = np.asarray(W, dtype=np.float32)
    a1 = np.asarray(a1, dtype=np.float32)
    a2 = np.asarray(a2, dtype=np.float32)
    c = np.asarray(c, dtype=np.float32)

    # projections + h on host (O(N*F) / 0.8% of kernel FLOPs)
    cs = x @ c                      # [N]
    ss = x @ (W @ a1)               # [N]
    sd = x @ (W @ a2)               # [N]
    h16 = np.ones((N, NMM), dtype=np.float16)
    h16[:, 0:F] = (x @ W).astype(np.float16)

    # per-j scalar columns [128, NJT, 3]: [K*sd+B, 0.2*K*sd+B, K*cs], j = t*128+p
    sd_t = sd.reshape(NJT, 128).T   # [128, NJT]
    cs_t = cs.reshape(NJT, 128).T
    scols = np.stack([K_SCALE * sd_t + B_BIAS,
                      0.2 * K_SCALE * sd_t + B_BIAS,
                      K_SCALE * cs_t], axis=2).astype(np.float32)

    # NaN-encoded mask: edge -> causal weight, non-edge -> NaN (the DVE MAX
    # suppresses NaN; LADD4's final MAX(,0) maps any masked residue to +0)
    cwm = np.where(adj > 0, cw, np.nan).astype(np.float16)

    in_maps = []
    for k in range(NCORES):
        r0, r1 = k * RPC, (k + 1) * RPC
        negcs = np.repeat((-K_SCALE * cs[r0:r1]).astype(np.float16)[None, :],
                          128, axis=0)
        ss2 = np.repeat((-0.8 * K_SCALE * ss[r0:r1]).astype(np.float16)[None, :],
                        128, axis=0)
        in_maps.append({
            "h16": h16,
            "cwmT": np.ascontiguousarray(cwm[r0:r1, :].T),
            "negcs": negcs,
            "ss2": ss2,
            "scols": scols,
        })
    return in_maps


def kernel(x, adj, causal_weights, W, a1, a2, c, _trace=False, _trace_kwargs=None):
    nc = _get_program()
    in_maps = _host_prep(x, adj, causal_weights, W, a1, a2, c)
    kw = {}
    if _trace:
        kw["trace"] = True
        kw.update(_trace_kwargs or {})
    res = run_bass_kernel_spmd(nc, in_maps, list(range(NCORES)), **kw)
    out = np.concatenate([res.results[k]["out"] for k in range(NCORES)], axis=0)

    # host check of the on-device 2x custom-op self-tests (core 0, jt=0)
    try:
        m0 = in_maps[0]
        negf = m0["negcs"][:, 0:128].astype(np.float32)
        cs_j = m0["scols"][:, 0, 2:3]
        cwm0 = m0["cwmT"][0:128, 0:128].astype(np.float32)
        w_exp = np.fmax((negf + cs_j) * cwm0, MASK_NEG)
        w_got = res.results[0]["dbg_w"].astype(np.float32)
        dw = np.abs(w_got - w_exp)
        print(f"[w3-2x selftest] max abs diff: {np.nanmax(dw):.4f}")
        ss2f = m0["ss2"][:, 0:128].astype(np.float32)
        i_exp = np.fmax(np.maximum(ss2f + m0["scols"][:, 0, 1:2],
                                   m0["scols"][:, 0, 0:1]) + w_got, 0.0)
        p_got = res.results[0]["dbg_p"].astype(np.int64)
        dp_ = np.abs(p_got - i_exp.astype(np.int64))
        print(f"[ladd4-2x selftest] max |diff|: {dp_.max()}")
    except Exception as exc:  # pragma: no cover - diagnostic only
        print(f"[2x selftest] check failed: {exc}")

    if _trace:
        return out, res
    return out
